# revision 34
# baseline (speedup 1.0000x reference)
"""MultiHuberLoss Trainium2 kernel (v10).

Reference (per element, with m = +x at the target class, -x elsewhere):
    hinge = max(0, 1 - m);  loss = where(m >= -1, hinge^2, -4m);  out = sum(loss)/N

Exact identities:
  F(-x) = (clamp(x,-1,1) + 1)^2 + 4*relu(x-1)          (main pass, all elements)
  F(x_t) - F(-x_t) = -4 * x_t                          (per-row target correction)
So:  sum(loss) = sum_ij (clamp+1)^2 + 4*[sum_ij relu(x-1) - sum_i x[i, t_i]]

Data parallel over 8 cores (8192 rows each).  Host-side, each core's rows
are SORTED BY TARGET and laid out so slot (p, j) holds sorted-rank j*128+p;
the 128 rows of a j-slot then share a narrow target column band, so one
variable-width (16-48 col) is_equal mask (scalar_tensor_tensor) per slot
extracts all 128 targets in ~250ns on DVE.  No gpsimd at all: the column-index ramp is a
host input (iota's first-use IRAM load costs ~6us of head latency).

Tiles taper at BOTH ends (1000,1000,2000,4000, 8000x6, 4000,2000,1000,1000)
so compute starts ~8us sooner and the serial v->Square tail stays short.
ACT writes are in-place (Square onto v, Relu onto the spent x tile) --
only the fused accumulators matter -- freeing SBUF for xp bufs=4.

Engine split per core:
  - DVE:  v = clamp(x,-1,1)->bf16 (2x fp32 mode); s = relu(x-1)->bf16 on
          PE-tiles; 64 is_equal extractions -> fp8 strip gs
  - ACT:  Square(v+1) accum -> accA; Relu(x-1) accum on two mid tiles
  - PE:   every "4*(B - G)" contribution accumulates into ONE PSUM region
          via ones^T (+1) / neg-ones^T (-1) matmuls on the in-order Tensor
          queue: + s chunks, + accB cols, - per-tile gs strip ranges
          (emitted as each tile's extractions finish, keeping the gather
          sum off the serial tail)
Epilogue: rA = rowsum(accA) -> psS = ones^T rA; rBG = rowsum(psB);
res = psS/N + 4*rBG/N (res allocated from the v ring: WAW-pinned late).
"""

import numpy as np

import concourse.bacc as bacc
import concourse.bass as bass
import concourse.mybir as mybir
from concourse.bass_utils import run_bass_kernel_spmd
from concourse.tile import TileContext

N_TOTAL = 65536
C = 1000
N_CORES = 8
ROWS = N_TOTAL // N_CORES  # 8192 rows per core
P = 128                    # partitions
JPP = ROWS // P            # 64 rows (slots) per partition
TILE_FD = [1000, 1000, 2000, 4000] + [8000] * 6 + [4000, 2000, 1000, 1000]
NT = len(TILE_FD)
W = 48                     # max is_equal window (per-slot widths in ws)
CHUNK = 500                # matmul rhs free-dim chunk

# tiles whose B-term (sum relu(x-1)) runs on ACT; the rest go DVE->PE
ACT_B_TILES = (5, 8)

f32 = mybir.dt.float32
bf16 = mybir.dt.bfloat16
fp8 = mybir.dt.float8e4
Alu = mybir.AluOpType


def build_program(c0s, ws):
    assert len(c0s) == JPP == len(ws)
    offs = [0]
    for w_ in ws:
        offs.append(offs[-1] + w_)
    assert sum(TILE_FD) == JPP * C
    nc = bacc.Bacc(
        "TRN2", target_bir_lowering=False, debug=False, num_devices=N_CORES
    )
    x = nc.dram_tensor("x", [ROWS, C], f32, kind="ExternalInput")
    tc_in = nc.dram_tensor("tc", [ROWS], f32, kind="ExternalInput")
    ci_in = nc.dram_tensor("ci", [P, C], f32, kind="ExternalInput")
    out = nc.dram_tensor("out", [1, 1], f32, kind="ExternalOutput")

    x_flat = x.ap().rearrange("(p j) c -> p (j c)", p=P)  # [128, 64000]
    tc2d = tc_in.ap().rearrange("(p j) -> p j", p=P)      # [128, 64]

    GS_W = offs[-1]  # fp8 strip width (sum of per-slot windows)

    with TileContext(nc) as tc:
        with (
            tc.tile_pool(name="xp", bufs=4) as xp,
            tc.tile_pool(name="vp", bufs=2) as vp,
            tc.tile_pool(name="sp", bufs=2) as sp,
            tc.tile_pool(name="small", bufs=1) as small,
            tc.tile_pool(name="psp", bufs=1, space="PSUM") as psp,
        ):
            ones_f = small.tile([P, 1], f32, tag="ones_f")
            nc.vector.memset(ones_f[:], 1.0)
            nones_f = small.tile([P, 1], f32, tag="nones_f")
            nc.vector.memset(nones_f[:], -1.0)
            ones = small.tile([P, 1], bf16, tag="ones")
            nc.vector.memset(ones[:], 1.0)
            nones8 = small.tile([P, 1], fp8, tag="nones8")
            nc.vector.memset(nones8[:], -1.0)
            ci = small.tile([P, C], f32, tag="ci")
            nc.sync.dma_start(out=ci[:], in_=ci_in.ap())
            tcv = small.tile([P, JPP], f32, tag="tcv")
            nc.sync.dma_start(out=tcv[:], in_=tc2d)

            accA = small.tile([P, NT], f32, tag="accA")
            accB = small.tile([P, len(ACT_B_TILES)], f32, tag="accB")
            # fp8 strip of masked target products, one W-window per slot
            gs = small.tile([P, GS_W], fp8, tag="gs")
            psB = psp.tile([1, CHUNK], f32, tag="psB")

            mm_first = True
            bcol = 0
            fd_off = 0
            for t, FD in enumerate(TILE_FD):
                RPT = FD // C
                j0 = fd_off // C
                xt = xp.tile([P, FD], f32)
                nc.sync.dma_start(
                    out=xt[:], in_=x_flat[:, fd_off:fd_off + FD]
                )
                v = vp.tile([P, FD], bf16)
                nc.vector.tensor_scalar(
                    v[:], xt[:], -1.0, 1.0, Alu.max, Alu.min
                )
                # in-place: squares overwrite v; only accA matters
                nc.scalar.activation(
                    v[:],
                    v[:],
                    mybir.ActivationFunctionType.Square,
                    bias=1.0,
                    scale=1.0,
                    accum_out=accA[:, t:t + 1],
                )
                if t not in ACT_B_TILES:
                    s = sp.tile([P, FD], bf16)
                    nc.vector.tensor_scalar(
                        s[:], xt[:], 1.0, 0.0, Alu.subtract, Alu.max
                    )
                    for k in range(FD // CHUNK):
                        nc.tensor.matmul(
                            out=psB[:],
                            lhsT=ones[:],
                            rhs=s[:, k * CHUNK:(k + 1) * CHUNK],
                            start=mm_first,
                            stop=False,
                            skip_group_check=True,
                        )
                        mm_first = False
                # target extraction: one W-wide is_equal scan per slot
                for jj in range(RPT):
                    gj = j0 + jj
                    c0 = c0s[gj]
                    wj = ws[gj]
                    o = offs[gj]
                    nc.vector.scalar_tensor_tensor(
                        out=gs[:, o:o + wj],
                        in0=ci[:, c0:c0 + wj],
                        scalar=tcv[:, gj:gj + 1],
                        in1=xt[:, jj * C + c0:jj * C + c0 + wj],
                        op0=Alu.is_equal,
                        op1=Alu.mult,
                    )
                # sum this tile's strip range into psB as soon as its
                # stt ops are done (keeps the gather sum off the tail)
                o0, o1 = offs[j0], offs[j0 + RPT]
                nc.tensor.matmul(
                    out=psB[:, 0:o1 - o0], lhsT=nones8[:],
                    rhs=gs[:, o0:o1],
                    start=False, stop=(t == NT - 1), skip_group_check=True,
                )
                if t in ACT_B_TILES:
                    # in-place onto the spent x tile (all readers are done;
                    # WAR deps order this after v/stt); only accB matters
                    nc.scalar.activation(
                        xt[:],
                        xt[:],
                        mybir.ActivationFunctionType.Relu,
                        bias=nones_f[:],
                        scale=1.0,
                        accum_out=accB[:, bcol:bcol + 1],
                    )
                    bcol += 1
                    if bcol == len(ACT_B_TILES):
                        nc.tensor.matmul(
                            out=psB[:, 0:len(ACT_B_TILES)], lhsT=ones_f[:],
                            rhs=accB[:], start=False, stop=False,
                            skip_group_check=True,
                        )
                fd_off += FD

            # ---- remaining contributions into the same PSUM region ----

            # ---- epilogue ----
            rA = small.tile([P, 1], f32, tag="rA")
            nc.vector.reduce_sum(rA[:], accA[:], axis=mybir.AxisListType.X)
            psS = psp.tile([1, 1], f32, tag="psS")
            nc.tensor.matmul(
                out=psS[:], lhsT=ones_f[:], rhs=rA[:], start=True, stop=True
            )
            rBG = small.tile([1, 1], f32, tag="rBG")
            nc.vector.reduce_sum(rBG[:], psB[:], axis=mybir.AxisListType.X)
            bias_t = small.tile([1, 1], f32, tag="bias_t")
            nc.vector.tensor_scalar(
                bias_t[:], rBG[:], 4.0 / N_TOTAL, None, Alu.mult
            )
            # allocate res from the v ring: WAW-pins it near the stream end
            # so the Scalar queue cannot hoist it
            res = vp.tile([1, 1], f32)
            nc.scalar.activation(
                res[:],
                psS[:],
                mybir.ActivationFunctionType.Identity,
                bias=bias_t[:],
                scale=1.0 / N_TOTAL,
            )
            nc.sync.dma_start(out=out.ap(), in_=res[:])

    nc.compile()
    return nc


_NC_CACHE = {}
LAST_RESULTS = None


def _prep(input, target):
    """Sort each core's rows by target; compute per-slot column bands."""
    x = np.asarray(input, dtype=np.float32)
    tg = np.asarray(target).astype(np.int64)
    xs, ts = [], []
    # slot (p, j) <- sorted rank j*128 + p;  dest row r = p*JPP + j
    r = np.arange(ROWS)
    k_of_r = (r % JPP) * P + r // JPP
    for c in range(N_CORES):
        t_c = tg[c * ROWS:(c + 1) * ROWS]
        order = np.argsort(t_c, kind="stable")
        perm = order[k_of_r]
        xs.append(np.ascontiguousarray(x[c * ROWS:(c + 1) * ROWS][perm]))
        ts.append(t_c[perm])
    tmat = np.stack(ts).reshape(N_CORES, P, JPP)
    tmin = tmat.min(axis=(0, 1))
    tmax = tmat.max(axis=(0, 1))
    spans = tmax - tmin + 1
    ws = np.maximum(((spans + 7) // 8) * 8, 16)
    c0s = np.clip(tmin, 0, C - ws)
    assert (tmax < c0s + ws).all(), (
        f"target band wider than window: spans {spans.max()}"
    )
    return xs, [t.astype(np.float32) for t in ts], \
        tuple(int(v) for v in c0s), tuple(int(v) for v in ws)


def kernel(input, target):
    global LAST_RESULTS
    x = np.asarray(input, dtype=np.float32)
    tg = np.asarray(target).astype(np.int64)
    assert x.shape == (N_TOTAL, C), x.shape
    assert tg.shape == (N_TOTAL,), tg.shape

    xs, ts, c0s, ws = _prep(x, tg)
    key = (c0s, ws)
    if key not in _NC_CACHE:
        _NC_CACHE[key] = build_program(c0s, ws)
    nc = _NC_CACHE[key]

    ci_host = np.ascontiguousarray(
        np.broadcast_to(np.arange(C, dtype=np.float32), (P, C))
    )
    in_maps = [
        {"x": xs[c], "tc": ts[c], "ci": ci_host} for c in range(N_CORES)
    ]
    res = run_bass_kernel_spmd(nc, in_maps, core_ids=list(range(N_CORES)))
    LAST_RESULTS = res
    total = np.float32(0.0)
    for r in res.results:
        total += np.float32(r["out"].reshape(()))
    return np.asarray(total, dtype=np.float32)


if __name__ == "__main__":
    rng = np.random.default_rng(0)
    xs = rng.standard_normal((N_TOTAL, C), dtype=np.float32)
    ts = rng.integers(0, C, size=(N_TOTAL,)).astype(np.int64)
    got = kernel(xs, ts)
    m = np.where(np.arange(C)[None, :] == ts[:, None], xs, -xs)
    hinge = np.maximum(0.0, 1.0 - m)
    loss = np.where(m >= -1.0, hinge * hinge, -4.0 * m)
    want = loss.sum(dtype=np.float64) / N_TOTAL
    print("got", got, "want", want, "rel", abs(got - want) / abs(want))


# revision 35
# speedup vs baseline: 1.0004x; 1.0004x over previous
"""MultiHuberLoss Trainium2 kernel (v10).

Reference (per element, with m = +x at the target class, -x elsewhere):
    hinge = max(0, 1 - m);  loss = where(m >= -1, hinge^2, -4m);  out = sum(loss)/N

Exact identities:
  F(-x) = (clamp(x,-1,1) + 1)^2 + 4*relu(x-1)          (main pass, all elements)
  F(x_t) - F(-x_t) = -4 * x_t                          (per-row target correction)
So:  sum(loss) = sum_ij (clamp+1)^2 + 4*[sum_ij relu(x-1) - sum_i x[i, t_i]]

Data parallel over 8 cores (8192 rows each).  Host-side, each core's rows
are SORTED BY TARGET and laid out so slot (p, j) holds sorted-rank j*128+p;
the 128 rows of a j-slot then share a narrow target column band, so one
variable-width (16-48 col) is_equal mask (scalar_tensor_tensor) per slot
extracts all 128 targets in ~250ns on DVE.  No gpsimd at all: the column-index ramp is a
host input (iota's first-use IRAM load costs ~6us of head latency).

Tiles taper at BOTH ends (1000,1000,2000,4000, 8000x6, 4000,2000,1000,1000)
so compute starts ~8us sooner and the serial v->Square tail stays short.
ACT writes are in-place (Square onto v, Relu onto the spent x tile) --
only the fused accumulators matter -- freeing SBUF for xp bufs=4.

Engine split per core:
  - DVE:  v = clamp(x,-1,1)->bf16 (2x fp32 mode); s = relu(x-1)->bf16 on
          PE-tiles; 64 is_equal extractions -> fp8 strip gs
  - ACT:  Square(v+1) accum -> accA; Relu(x-1) accum on two mid tiles
  - PE:   every "4*(B - G)" contribution accumulates into ONE PSUM region
          via ones^T (+1) / neg-ones^T (-1) matmuls on the in-order Tensor
          queue: + s chunks, + accB cols, - per-tile gs strip ranges
          (emitted as each tile's extractions finish, keeping the gather
          sum off the serial tail)
Epilogue: rA = rowsum(accA) -> psS = ones^T rA; rBG = rowsum(psB);
res = psS/N + 4*rBG/N (res allocated from the v ring: WAW-pinned late).
"""

import numpy as np

import concourse.bacc as bacc
import concourse.bass as bass
import concourse.mybir as mybir
from concourse.bass_utils import run_bass_kernel_spmd
from concourse.tile import TileContext

N_TOTAL = 65536
C = 1000
N_CORES = 8
ROWS = N_TOTAL // N_CORES  # 8192 rows per core
P = 128                    # partitions
JPP = ROWS // P            # 64 rows (slots) per partition
TILE_FD = [1000, 1000, 2000, 4000] + [8000] * 6 + [4000, 2000, 1000, 1000]
NT = len(TILE_FD)
W = 48                     # max is_equal window (per-slot widths in ws)
CHUNK = 500                # matmul rhs free-dim chunk

# tiles whose B-term (sum relu(x-1)) runs on ACT; the rest go DVE->PE
ACT_B_TILES = (5, 8)

f32 = mybir.dt.float32
bf16 = mybir.dt.bfloat16
fp8 = mybir.dt.float8e4
Alu = mybir.AluOpType


def build_program(c0s, ws):
    assert len(c0s) == JPP == len(ws)
    offs = [0]
    for w_ in ws:
        offs.append(offs[-1] + w_)
    assert sum(TILE_FD) == JPP * C
    nc = bacc.Bacc(
        "TRN2", target_bir_lowering=False, debug=False, num_devices=N_CORES
    )
    x = nc.dram_tensor("x", [ROWS, C], f32, kind="ExternalInput")
    tc_in = nc.dram_tensor("tc", [ROWS], f32, kind="ExternalInput")
    ci_in = nc.dram_tensor("ci", [P, C], f32, kind="ExternalInput")
    out = nc.dram_tensor("out", [1, 1], f32, kind="ExternalOutput")

    x_flat = x.ap().rearrange("(p j) c -> p (j c)", p=P)  # [128, 64000]
    tc2d = tc_in.ap().rearrange("(p j) -> p j", p=P)      # [128, 64]

    GS_W = offs[-1]  # fp8 strip width (sum of per-slot windows)

    with TileContext(nc) as tc:
        with (
            tc.tile_pool(name="xp", bufs=4) as xp,
            tc.tile_pool(name="vp", bufs=3) as vp,
            tc.tile_pool(name="sp", bufs=2) as sp,
            tc.tile_pool(name="small", bufs=1) as small,
            tc.tile_pool(name="psp", bufs=1, space="PSUM") as psp,
        ):
            ones_f = small.tile([P, 1], f32, tag="ones_f")
            nc.vector.memset(ones_f[:], 1.0)
            nones_f = small.tile([P, 1], f32, tag="nones_f")
            nc.vector.memset(nones_f[:], -1.0)
            ones = small.tile([P, 1], bf16, tag="ones")
            nc.vector.memset(ones[:], 1.0)
            nones8 = small.tile([P, 1], fp8, tag="nones8")
            nc.vector.memset(nones8[:], -1.0)
            ones8 = small.tile([P, 1], fp8, tag="ones8")
            nc.vector.memset(ones8[:], 1.0)
            ci = small.tile([P, C], f32, tag="ci")
            nc.sync.dma_start(out=ci[:], in_=ci_in.ap())
            tcv = small.tile([P, JPP], f32, tag="tcv")
            nc.sync.dma_start(out=tcv[:], in_=tc2d)

            accA = small.tile([P, NT], f32, tag="accA")
            accB = small.tile([P, len(ACT_B_TILES)], f32, tag="accB")
            # fp8 strip of masked target products, one W-window per slot
            gs = small.tile([P, GS_W], fp8, tag="gs")
            psB = psp.tile([1, CHUNK], f32, tag="psB")

            mm_first = True
            bcol = 0
            fd_off = 0
            for t, FD in enumerate(TILE_FD):
                RPT = FD // C
                j0 = fd_off // C
                xt = xp.tile([P, FD], f32)
                nc.sync.dma_start(
                    out=xt[:], in_=x_flat[:, fd_off:fd_off + FD]
                )
                v = vp.tile([P, FD], bf16)
                nc.vector.tensor_scalar(
                    v[:], xt[:], -1.0, 1.0, Alu.max, Alu.min
                )
                # in-place: squares overwrite v; only accA matters
                nc.scalar.activation(
                    v[:],
                    v[:],
                    mybir.ActivationFunctionType.Square,
                    bias=1.0,
                    scale=1.0,
                    accum_out=accA[:, t:t + 1],
                )
                if t not in ACT_B_TILES:
                    s = sp.tile([P, FD], fp8)
                    nc.vector.tensor_scalar(
                        s[:], xt[:], 1.0, 0.0, Alu.subtract, Alu.max
                    )
                    for k in range(FD // CHUNK):
                        nc.tensor.matmul(
                            out=psB[:],
                            lhsT=ones8[:],
                            rhs=s[:, k * CHUNK:(k + 1) * CHUNK],
                            start=mm_first,
                            stop=False,
                            skip_group_check=True,
                        )
                        mm_first = False
                # target extraction: one W-wide is_equal scan per slot
                for jj in range(RPT):
                    gj = j0 + jj
                    c0 = c0s[gj]
                    wj = ws[gj]
                    o = offs[gj]
                    nc.vector.scalar_tensor_tensor(
                        out=gs[:, o:o + wj],
                        in0=ci[:, c0:c0 + wj],
                        scalar=tcv[:, gj:gj + 1],
                        in1=xt[:, jj * C + c0:jj * C + c0 + wj],
                        op0=Alu.is_equal,
                        op1=Alu.mult,
                    )
                # sum this tile's strip range into psB as soon as its
                # stt ops are done (keeps the gather sum off the tail)
                o0, o1 = offs[j0], offs[j0 + RPT]
                nc.tensor.matmul(
                    out=psB[:, 0:o1 - o0], lhsT=nones8[:],
                    rhs=gs[:, o0:o1],
                    start=False, stop=(t == NT - 1), skip_group_check=True,
                )
                if t in ACT_B_TILES:
                    # in-place onto the spent x tile (all readers are done;
                    # WAR deps order this after v/stt); only accB matters
                    nc.scalar.activation(
                        xt[:],
                        xt[:],
                        mybir.ActivationFunctionType.Relu,
                        bias=nones_f[:],
                        scale=1.0,
                        accum_out=accB[:, bcol:bcol + 1],
                    )
                    bcol += 1
                    if bcol == len(ACT_B_TILES):
                        nc.tensor.matmul(
                            out=psB[:, 0:len(ACT_B_TILES)], lhsT=ones_f[:],
                            rhs=accB[:], start=False, stop=False,
                            skip_group_check=True,
                        )
                fd_off += FD

            # ---- remaining contributions into the same PSUM region ----

            # ---- epilogue ----
            rA = small.tile([P, 1], f32, tag="rA")
            nc.vector.reduce_sum(rA[:], accA[:], axis=mybir.AxisListType.X)
            psS = psp.tile([1, 1], f32, tag="psS")
            nc.tensor.matmul(
                out=psS[:], lhsT=ones_f[:], rhs=rA[:], start=True, stop=True
            )
            rBG = small.tile([1, 1], f32, tag="rBG")
            nc.vector.reduce_sum(rBG[:], psB[:], axis=mybir.AxisListType.X)
            bias_t = small.tile([1, 1], f32, tag="bias_t")
            nc.vector.tensor_scalar(
                bias_t[:], rBG[:], 4.0 / N_TOTAL, None, Alu.mult
            )
            # allocate res from the v ring: WAW-pins it near the stream end
            # so the Scalar queue cannot hoist it
            res = vp.tile([1, 1], f32)
            nc.scalar.activation(
                res[:],
                psS[:],
                mybir.ActivationFunctionType.Identity,
                bias=bias_t[:],
                scale=1.0 / N_TOTAL,
            )
            nc.sync.dma_start(out=out.ap(), in_=res[:])

    nc.compile()
    return nc


_NC_CACHE = {}
LAST_RESULTS = None


def _prep(input, target):
    """Sort each core's rows by target; compute per-slot column bands."""
    x = np.asarray(input, dtype=np.float32)
    tg = np.asarray(target).astype(np.int64)
    xs, ts = [], []
    # slot (p, j) <- sorted rank j*128 + p;  dest row r = p*JPP + j
    r = np.arange(ROWS)
    k_of_r = (r % JPP) * P + r // JPP
    for c in range(N_CORES):
        t_c = tg[c * ROWS:(c + 1) * ROWS]
        order = np.argsort(t_c, kind="stable")
        perm = order[k_of_r]
        xs.append(np.ascontiguousarray(x[c * ROWS:(c + 1) * ROWS][perm]))
        ts.append(t_c[perm])
    tmat = np.stack(ts).reshape(N_CORES, P, JPP)
    tmin = tmat.min(axis=(0, 1))
    tmax = tmat.max(axis=(0, 1))
    spans = tmax - tmin + 1
    ws = np.maximum(((spans + 7) // 8) * 8, 16)
    c0s = np.clip(tmin, 0, C - ws)
    assert (tmax < c0s + ws).all(), (
        f"target band wider than window: spans {spans.max()}"
    )
    return xs, [t.astype(np.float32) for t in ts], \
        tuple(int(v) for v in c0s), tuple(int(v) for v in ws)


def kernel(input, target):
    global LAST_RESULTS
    x = np.asarray(input, dtype=np.float32)
    tg = np.asarray(target).astype(np.int64)
    assert x.shape == (N_TOTAL, C), x.shape
    assert tg.shape == (N_TOTAL,), tg.shape

    xs, ts, c0s, ws = _prep(x, tg)
    key = (c0s, ws)
    if key not in _NC_CACHE:
        _NC_CACHE[key] = build_program(c0s, ws)
    nc = _NC_CACHE[key]

    ci_host = np.ascontiguousarray(
        np.broadcast_to(np.arange(C, dtype=np.float32), (P, C))
    )
    in_maps = [
        {"x": xs[c], "tc": ts[c], "ci": ci_host} for c in range(N_CORES)
    ]
    res = run_bass_kernel_spmd(nc, in_maps, core_ids=list(range(N_CORES)))
    LAST_RESULTS = res
    total = np.float32(0.0)
    for r in res.results:
        total += np.float32(r["out"].reshape(()))
    return np.asarray(total, dtype=np.float32)


if __name__ == "__main__":
    rng = np.random.default_rng(0)
    xs = rng.standard_normal((N_TOTAL, C), dtype=np.float32)
    ts = rng.integers(0, C, size=(N_TOTAL,)).astype(np.int64)
    got = kernel(xs, ts)
    m = np.where(np.arange(C)[None, :] == ts[:, None], xs, -xs)
    hinge = np.maximum(0.0, 1.0 - m)
    loss = np.where(m >= -1.0, hinge * hinge, -4.0 * m)
    want = loss.sum(dtype=np.float64) / N_TOTAL
    print("got", got, "want", want, "rel", abs(got - want) / abs(want))


# revision 37
# speedup vs baseline: 1.0042x; 1.0038x over previous
"""MultiHuberLoss Trainium2 kernel (v13).

Reference (per element, with m = +x at the target class, -x elsewhere):
    hinge = max(0, 1 - m);  loss = where(m >= -1, hinge^2, -4m);  out = sum(loss)/N

Exact identities:
  F(-x) = (clamp(x,-1,1) + 1)^2 + 4*relu(x-1)          (main pass, all elements)
  F(x_t) - F(-x_t) = -4 * x_t                          (per-row target correction)
So:  sum(loss) = sum_ij (clamp+1)^2 + 4*[sum_ij relu(x-1) - sum_i x[i, t_i]]

Data parallel over 8 cores (8192 rows each).  Host-side, each core's rows
are SORTED BY TARGET and laid out so slot (p, j) holds sorted-rank j*128+p;
the 128 rows of a j-slot then share a narrow target column band, so one
variable-width (16-48 col) is_equal mask (scalar_tensor_tensor) per slot
extracts all 128 targets in ~250ns on DVE.  No gpsimd at all: the column-index ramp is a
host input (iota's first-use IRAM load costs ~6us of head latency).

Tiles taper at BOTH ends (1000,1000,2000,4000, 8000x6, 4000,2000,1000,1000)
so compute starts ~8us sooner and the serial v->Square tail stays short.
ACT writes are in-place (Square onto v, Relu onto the spent x tile) --
only the fused accumulators matter -- freeing SBUF for xp bufs=4.

Engine split per core:
  - DVE:  v = clamp(x,-1,1)->bf16 (2x fp32 mode); s = relu(x-1)->fp8 on
          PE-tiles (fp8 frees SBUF for the 3-deep v ring); 64 is_equal
          extractions -> fp8 strip gs
  - ACT:  Square(v+1) accum -> accA; Relu(x-1) accum on two mid tiles
  - PE:   every "4*(B - G)" contribution accumulates into ONE PSUM region
          via ones^T (+1) / neg-ones^T (-1) matmuls on the in-order Tensor
          queue: + s chunks, + accB cols, - per-tile gs strip ranges
          (emitted as each tile's extractions finish, keeping the gather
          sum off the serial tail)
Epilogue: rA = rowsum(accA) -> psS = ones^T rA; rBG = rowsum(psB);
res = psS/N + 4*rBG/N (res allocated from the v ring: WAW-pinned late).
"""

import numpy as np

import concourse.bacc as bacc
import concourse.bass as bass
import concourse.mybir as mybir
from concourse.bass_utils import run_bass_kernel_spmd
from concourse.tile import TileContext

N_TOTAL = 65536
C = 1000
N_CORES = 8
ROWS = N_TOTAL // N_CORES  # 8192 rows per core
P = 128                    # partitions
JPP = ROWS // P            # 64 rows (slots) per partition
TILE_FD = [1000, 1000, 2000, 4000] + [8000] * 6 + [4000, 2000, 1000, 1000]
NT = len(TILE_FD)
W = 48                     # max is_equal window (per-slot widths in ws)
CHUNK = 500                # matmul rhs free-dim chunk

# tiles whose B-term (sum relu(x-1)) runs on ACT; the rest go DVE->PE
ACT_B_TILES = (5, 8)

f32 = mybir.dt.float32
bf16 = mybir.dt.bfloat16
fp8 = mybir.dt.float8e4
Alu = mybir.AluOpType


def build_program(c0s, ws):
    assert len(c0s) == JPP == len(ws)
    offs = [0]
    for w_ in ws:
        offs.append(offs[-1] + w_)
    assert sum(TILE_FD) == JPP * C
    nc = bacc.Bacc(
        "TRN2", target_bir_lowering=False, debug=False, num_devices=N_CORES
    )
    x = nc.dram_tensor("x", [ROWS, C], f32, kind="ExternalInput")
    tc_in = nc.dram_tensor("tc", [ROWS], f32, kind="ExternalInput")
    ci_in = nc.dram_tensor("ci", [P, C], f32, kind="ExternalInput")
    out = nc.dram_tensor("out", [1, 1], f32, kind="ExternalOutput")

    x_flat = x.ap().rearrange("(p j) c -> p (j c)", p=P)  # [128, 64000]
    tc2d = tc_in.ap().rearrange("(p j) -> p j", p=P)      # [128, 64]

    GS_W = offs[-1]  # fp8 strip width (sum of per-slot windows)

    with TileContext(nc) as tc:
        with (
            tc.tile_pool(name="xp", bufs=5) as xp,
            tc.tile_pool(name="vp", bufs=3) as vp,
            tc.tile_pool(name="sp", bufs=2) as sp,
            tc.tile_pool(name="small", bufs=1) as small,
            tc.tile_pool(name="psp", bufs=1, space="PSUM") as psp,
        ):
            ones_f = small.tile([P, 1], f32, tag="ones_f")
            nc.vector.memset(ones_f[:], 1.0)
            nones_f = small.tile([P, 1], f32, tag="nones_f")
            nc.vector.memset(nones_f[:], -1.0)
            ones = small.tile([P, 1], bf16, tag="ones")
            nc.vector.memset(ones[:], 1.0)
            nones8 = small.tile([P, 1], fp8, tag="nones8")
            nc.vector.memset(nones8[:], -1.0)
            ones8 = small.tile([P, 1], fp8, tag="ones8")
            nc.vector.memset(ones8[:], 1.0)
            ci = small.tile([P, C], f32, tag="ci")
            nc.sync.dma_start(out=ci[:], in_=ci_in.ap())
            tcv = small.tile([P, JPP], f32, tag="tcv")
            nc.sync.dma_start(out=tcv[:], in_=tc2d)

            accA = small.tile([P, NT], f32, tag="accA")
            accB = small.tile([P, len(ACT_B_TILES)], f32, tag="accB")
            # fp8 strip of masked target products, one W-window per slot
            gs = small.tile([P, GS_W], fp8, tag="gs")
            psB = psp.tile([1, CHUNK], f32, tag="psB")

            mm_first = True
            bcol = 0
            fd_off = 0
            for t, FD in enumerate(TILE_FD):
                RPT = FD // C
                j0 = fd_off // C
                xt = xp.tile([P, FD], f32)
                nc.sync.dma_start(
                    out=xt[:], in_=x_flat[:, fd_off:fd_off + FD]
                )
                v = vp.tile([P, FD], fp8)
                nc.vector.tensor_scalar(
                    v[:], xt[:], -1.0, 1.0, Alu.max, Alu.min
                )
                # in-place: squares overwrite v; only accA matters
                nc.scalar.activation(
                    v[:],
                    v[:],
                    mybir.ActivationFunctionType.Square,
                    bias=1.0,
                    scale=1.0,
                    accum_out=accA[:, t:t + 1],
                )
                if t not in ACT_B_TILES:
                    s = sp.tile([P, FD], fp8)
                    nc.vector.tensor_scalar(
                        s[:], xt[:], 1.0, 0.0, Alu.subtract, Alu.max
                    )
                    for k in range(FD // CHUNK):
                        nc.tensor.matmul(
                            out=psB[:],
                            lhsT=ones8[:],
                            rhs=s[:, k * CHUNK:(k + 1) * CHUNK],
                            start=mm_first,
                            stop=False,
                            skip_group_check=True,
                        )
                        mm_first = False
                # target extraction: one W-wide is_equal scan per slot
                for jj in range(RPT):
                    gj = j0 + jj
                    c0 = c0s[gj]
                    wj = ws[gj]
                    o = offs[gj]
                    nc.vector.scalar_tensor_tensor(
                        out=gs[:, o:o + wj],
                        in0=ci[:, c0:c0 + wj],
                        scalar=tcv[:, gj:gj + 1],
                        in1=xt[:, jj * C + c0:jj * C + c0 + wj],
                        op0=Alu.is_equal,
                        op1=Alu.mult,
                    )
                # sum this tile's strip range into psB as soon as its
                # stt ops are done (keeps the gather sum off the tail)
                o0, o1 = offs[j0], offs[j0 + RPT]
                nc.tensor.matmul(
                    out=psB[:, 0:o1 - o0], lhsT=nones8[:],
                    rhs=gs[:, o0:o1],
                    start=False, stop=(t == NT - 1), skip_group_check=True,
                )
                if t in ACT_B_TILES:
                    # in-place onto the spent x tile (all readers are done;
                    # WAR deps order this after v/stt); only accB matters
                    nc.scalar.activation(
                        xt[:],
                        xt[:],
                        mybir.ActivationFunctionType.Relu,
                        bias=nones_f[:],
                        scale=1.0,
                        accum_out=accB[:, bcol:bcol + 1],
                    )
                    bcol += 1
                    if bcol == len(ACT_B_TILES):
                        nc.tensor.matmul(
                            out=psB[:, 0:len(ACT_B_TILES)], lhsT=ones_f[:],
                            rhs=accB[:], start=False, stop=False,
                            skip_group_check=True,
                        )
                fd_off += FD

            # ---- remaining contributions into the same PSUM region ----

            # ---- epilogue ----
            rA = small.tile([P, 1], f32, tag="rA")
            nc.vector.reduce_sum(rA[:], accA[:], axis=mybir.AxisListType.X)
            psS = psp.tile([1, 1], f32, tag="psS")
            nc.tensor.matmul(
                out=psS[:], lhsT=ones_f[:], rhs=rA[:], start=True, stop=True
            )
            rBG = small.tile([1, 1], f32, tag="rBG")
            nc.vector.reduce_sum(rBG[:], psB[:], axis=mybir.AxisListType.X)
            bias_t = small.tile([1, 1], f32, tag="bias_t")
            nc.vector.tensor_scalar(
                bias_t[:], rBG[:], 4.0 / N_TOTAL, None, Alu.mult
            )
            # allocate res from the v ring: WAW-pins it near the stream end
            # so the Scalar queue cannot hoist it
            res = vp.tile([1, 1], f32)
            nc.scalar.activation(
                res[:],
                psS[:],
                mybir.ActivationFunctionType.Identity,
                bias=bias_t[:],
                scale=1.0 / N_TOTAL,
            )
            nc.sync.dma_start(out=out.ap(), in_=res[:])

    nc.compile()
    return nc


_NC_CACHE = {}
LAST_RESULTS = None


def _prep(input, target):
    """Sort each core's rows by target; compute per-slot column bands."""
    x = np.asarray(input, dtype=np.float32)
    tg = np.asarray(target).astype(np.int64)
    xs, ts = [], []
    # slot (p, j) <- sorted rank j*128 + p;  dest row r = p*JPP + j
    r = np.arange(ROWS)
    k_of_r = (r % JPP) * P + r // JPP
    for c in range(N_CORES):
        t_c = tg[c * ROWS:(c + 1) * ROWS]
        order = np.argsort(t_c, kind="stable")
        perm = order[k_of_r]
        xs.append(np.ascontiguousarray(x[c * ROWS:(c + 1) * ROWS][perm]))
        ts.append(t_c[perm])
    tmat = np.stack(ts).reshape(N_CORES, P, JPP)
    tmin = tmat.min(axis=(0, 1))
    tmax = tmat.max(axis=(0, 1))
    spans = tmax - tmin + 1
    ws = np.maximum(((spans + 7) // 8) * 8, 16)
    c0s = np.clip(tmin, 0, C - ws)
    assert (tmax < c0s + ws).all(), (
        f"target band wider than window: spans {spans.max()}"
    )
    return xs, [t.astype(np.float32) for t in ts], \
        tuple(int(v) for v in c0s), tuple(int(v) for v in ws)


def kernel(input, target):
    global LAST_RESULTS
    x = np.asarray(input, dtype=np.float32)
    tg = np.asarray(target).astype(np.int64)
    assert x.shape == (N_TOTAL, C), x.shape
    assert tg.shape == (N_TOTAL,), tg.shape

    xs, ts, c0s, ws = _prep(x, tg)
    key = (c0s, ws)
    if key not in _NC_CACHE:
        _NC_CACHE[key] = build_program(c0s, ws)
    nc = _NC_CACHE[key]

    ci_host = np.ascontiguousarray(
        np.broadcast_to(np.arange(C, dtype=np.float32), (P, C))
    )
    in_maps = [
        {"x": xs[c], "tc": ts[c], "ci": ci_host} for c in range(N_CORES)
    ]
    res = run_bass_kernel_spmd(nc, in_maps, core_ids=list(range(N_CORES)))
    LAST_RESULTS = res
    total = np.float32(0.0)
    for r in res.results:
        total += np.float32(r["out"].reshape(()))
    return np.asarray(total, dtype=np.float32)


if __name__ == "__main__":
    rng = np.random.default_rng(0)
    xs = rng.standard_normal((N_TOTAL, C), dtype=np.float32)
    ts = rng.integers(0, C, size=(N_TOTAL,)).astype(np.int64)
    got = kernel(xs, ts)
    m = np.where(np.arange(C)[None, :] == ts[:, None], xs, -xs)
    hinge = np.maximum(0.0, 1.0 - m)
    loss = np.where(m >= -1.0, hinge * hinge, -4.0 * m)
    want = loss.sum(dtype=np.float64) / N_TOTAL
    print("got", got, "want", want, "rel", abs(got - want) / abs(want))


# revision 39
# speedup vs baseline: 1.0079x; 1.0036x over previous
"""MultiHuberLoss Trainium2 kernel (v14).

Reference (per element, with m = +x at the target class, -x elsewhere):
    hinge = max(0, 1 - m);  loss = where(m >= -1, hinge^2, -4m);  out = sum(loss)/N

Exact identities:
  F(-x) = (clamp(x,-1,1) + 1)^2 + 4*relu(x-1)          (main pass, all elements)
  F(x_t) - F(-x_t) = -4 * x_t                          (per-row target correction)
So:  sum(loss) = sum_ij (clamp+1)^2 + 4*[sum_ij relu(x-1) - sum_i x[i, t_i]]

Data parallel over 8 cores (8192 rows each).  Host-side, each core's rows
are SORTED BY TARGET and laid out so slot (p, j) holds sorted-rank j*128+p;
the 128 rows of a j-slot then share a narrow target column band, so one
variable-width (16-48 col) is_equal mask (scalar_tensor_tensor) per slot
extracts all 128 targets in ~250ns on DVE.  No gpsimd at all: the column-index ramp is a
host input (iota's first-use IRAM load costs ~6us of head latency).

Tiles taper at BOTH ends (1000,1000,2000,4000, 8000x6, 4000,2000,1000,1000)
so compute starts ~8us sooner and the serial v->Square tail stays short.
ACT writes are in-place (Square onto v, Relu onto the spent x tile) --
only the fused accumulators matter -- freeing SBUF for xp bufs=5.

Engine split per core:
  - DVE:  v = clamp(x,-1,1)->fp8 (2x fp32 mode); s = relu(x-1)->fp8 on
          PE-tiles (fp8 intermediates free SBUF for the 3-deep v ring and
          a 5-deep x prefetch); 64 is_equal extractions -> fp8 strip gs
  - ACT:  Square(v+1) accum -> accA; Relu(x-1) accum on two mid tiles
  - PE:   every "4*(B - G)" contribution accumulates into ONE PSUM region
          via ones^T (+1) / neg-ones^T (-1) matmuls on the in-order Tensor
          queue: + s chunks, + accB cols, - per-tile gs strip ranges
          (emitted as each tile's extractions finish, keeping the gather
          sum off the serial tail)
Epilogue: rA = rowsum(accA) -> psS = ones^T rA; rBG = rowsum(psB);
res = psS/N + 4*rBG/N (res allocated from the v ring: WAW-pinned late).
"""

import numpy as np

import concourse.bacc as bacc
import concourse.bass as bass
import concourse.mybir as mybir
from concourse.bass_utils import run_bass_kernel_spmd
from concourse.tile import TileContext

N_TOTAL = 65536
C = 1000
N_CORES = 8
ROWS = N_TOTAL // N_CORES  # 8192 rows per core
P = 128                    # partitions
JPP = ROWS // P            # 64 rows (slots) per partition
TILE_FD = [1000, 1000, 2000, 4000] + [8000] * 6 + [4000, 2000, 1000, 1000]
NT = len(TILE_FD)
W = 48                     # max is_equal window (per-slot widths in ws)
CHUNK = 500                # matmul rhs free-dim chunk

# tiles whose B-term (sum relu(x-1)) runs on ACT; the rest go DVE->PE
ACT_B_TILES = (5, 8)

f32 = mybir.dt.float32
bf16 = mybir.dt.bfloat16
fp8 = mybir.dt.float8e4
Alu = mybir.AluOpType


def build_program(c0s, ws):
    assert len(c0s) == JPP == len(ws)
    offs = [0]
    for w_ in ws:
        offs.append(offs[-1] + w_)
    assert sum(TILE_FD) == JPP * C
    nc = bacc.Bacc(
        "TRN2", target_bir_lowering=False, debug=False, num_devices=N_CORES
    )
    x = nc.dram_tensor("x", [ROWS, C], f32, kind="ExternalInput")
    tc_in = nc.dram_tensor("tc", [ROWS], f32, kind="ExternalInput")
    ci_in = nc.dram_tensor("ci", [P, C], f32, kind="ExternalInput")
    out = nc.dram_tensor("out", [1, 1], f32, kind="ExternalOutput")

    x_flat = x.ap().rearrange("(p j) c -> p (j c)", p=P)  # [128, 64000]
    tc2d = tc_in.ap().rearrange("(p j) -> p j", p=P)      # [128, 64]

    GS_W = offs[-1]  # fp8 strip width (sum of per-slot windows)

    with TileContext(nc) as tc:
        with (
            tc.tile_pool(name="xp", bufs=5) as xp,
            tc.tile_pool(name="vp", bufs=3) as vp,
            tc.tile_pool(name="sp", bufs=2) as sp,
            tc.tile_pool(name="small", bufs=1) as small,
            tc.tile_pool(name="psp", bufs=1, space="PSUM") as psp,
        ):
            ones_f = small.tile([P, 1], f32, tag="ones_f")
            nc.vector.memset(ones_f[:], 1.0)
            nones_f = small.tile([P, 1], f32, tag="nones_f")
            nc.vector.memset(nones_f[:], -1.0)
            nones8 = small.tile([P, 1], fp8, tag="nones8")
            nc.vector.memset(nones8[:], -1.0)
            ones8 = small.tile([P, 1], fp8, tag="ones8")
            nc.vector.memset(ones8[:], 1.0)
            ci = small.tile([P, C], f32, tag="ci")
            nc.sync.dma_start(out=ci[:], in_=ci_in.ap())
            tcv = small.tile([P, JPP], f32, tag="tcv")
            nc.sync.dma_start(out=tcv[:], in_=tc2d)

            accA = small.tile([P, NT], f32, tag="accA")
            accB = small.tile([P, len(ACT_B_TILES)], f32, tag="accB")
            # fp8 strip of masked target products, one W-window per slot
            gs = small.tile([P, GS_W], fp8, tag="gs")
            psB = psp.tile([1, CHUNK], f32, tag="psB")

            mm_first = True
            bcol = 0
            fd_off = 0
            for t, FD in enumerate(TILE_FD):
                RPT = FD // C
                j0 = fd_off // C
                xt = xp.tile([P, FD], f32)
                nc.sync.dma_start(
                    out=xt[:], in_=x_flat[:, fd_off:fd_off + FD]
                )
                v = vp.tile([P, FD], fp8)
                nc.vector.tensor_scalar(
                    v[:], xt[:], -1.0, 1.0, Alu.max, Alu.min
                )
                # in-place: squares overwrite v; only accA matters
                nc.scalar.activation(
                    v[:],
                    v[:],
                    mybir.ActivationFunctionType.Square,
                    bias=1.0,
                    scale=1.0,
                    accum_out=accA[:, t:t + 1],
                )
                if t not in ACT_B_TILES:
                    s = sp.tile([P, FD], fp8)
                    nc.vector.tensor_scalar(
                        s[:], xt[:], 1.0, 0.0, Alu.subtract, Alu.max
                    )
                    for k in range(FD // CHUNK):
                        nc.tensor.matmul(
                            out=psB[:],
                            lhsT=ones8[:],
                            rhs=s[:, k * CHUNK:(k + 1) * CHUNK],
                            start=mm_first,
                            stop=False,
                            skip_group_check=True,
                        )
                        mm_first = False
                # target extraction: one W-wide is_equal scan per slot
                for jj in range(RPT):
                    gj = j0 + jj
                    c0 = c0s[gj]
                    wj = ws[gj]
                    o = offs[gj]
                    nc.vector.scalar_tensor_tensor(
                        out=gs[:, o:o + wj],
                        in0=ci[:, c0:c0 + wj],
                        scalar=tcv[:, gj:gj + 1],
                        in1=xt[:, jj * C + c0:jj * C + c0 + wj],
                        op0=Alu.is_equal,
                        op1=Alu.mult,
                    )
                # sum this tile's strip range into psB as soon as its
                # stt ops are done (keeps the gather sum off the tail)
                o0, o1 = offs[j0], offs[j0 + RPT]
                nc.tensor.matmul(
                    out=psB[:, 0:o1 - o0], lhsT=nones8[:],
                    rhs=gs[:, o0:o1],
                    start=False, stop=(t == NT - 1), skip_group_check=True,
                )
                if t in ACT_B_TILES:
                    # in-place onto the spent x tile (all readers are done;
                    # WAR deps order this after v/stt); only accB matters
                    nc.scalar.activation(
                        xt[:],
                        xt[:],
                        mybir.ActivationFunctionType.Relu,
                        bias=nones_f[:],
                        scale=1.0,
                        accum_out=accB[:, bcol:bcol + 1],
                    )
                    bcol += 1
                    if bcol == len(ACT_B_TILES):
                        nc.tensor.matmul(
                            out=psB[:, 0:len(ACT_B_TILES)], lhsT=ones_f[:],
                            rhs=accB[:], start=False, stop=False,
                            skip_group_check=True,
                        )
                fd_off += FD

            # ---- remaining contributions into the same PSUM region ----

            # ---- epilogue ----
            rA = small.tile([P, 1], f32, tag="rA")
            nc.vector.reduce_sum(rA[:], accA[:], axis=mybir.AxisListType.X)
            psS = psp.tile([1, 1], f32, tag="psS")
            nc.tensor.matmul(
                out=psS[:], lhsT=ones_f[:], rhs=rA[:], start=True, stop=True
            )
            rBG = small.tile([1, 1], f32, tag="rBG")
            nc.vector.reduce_sum(rBG[:], psB[:], axis=mybir.AxisListType.X)
            bias_t = small.tile([1, 1], f32, tag="bias_t")
            nc.vector.tensor_scalar(
                bias_t[:], rBG[:], 4.0 / N_TOTAL, None, Alu.mult
            )
            # allocate res from the v ring: WAW-pins it near the stream end
            # so the Scalar queue cannot hoist it
            res = vp.tile([1, 1], f32)
            nc.scalar.activation(
                res[:],
                psS[:],
                mybir.ActivationFunctionType.Identity,
                bias=bias_t[:],
                scale=1.0 / N_TOTAL,
            )
            nc.sync.dma_start(out=out.ap(), in_=res[:])

    nc.compile()
    return nc


_NC_CACHE = {}
LAST_RESULTS = None


def _prep(input, target):
    """Sort each core's rows by target; compute per-slot column bands."""
    x = np.asarray(input, dtype=np.float32)
    tg = np.asarray(target).astype(np.int64)
    xs, ts = [], []
    # slot (p, j) <- sorted rank j*128 + p;  dest row r = p*JPP + j
    r = np.arange(ROWS)
    k_of_r = (r % JPP) * P + r // JPP
    for c in range(N_CORES):
        t_c = tg[c * ROWS:(c + 1) * ROWS]
        order = np.argsort(t_c, kind="stable")
        perm = order[k_of_r]
        xs.append(np.ascontiguousarray(x[c * ROWS:(c + 1) * ROWS][perm]))
        ts.append(t_c[perm])
    tmat = np.stack(ts).reshape(N_CORES, P, JPP)
    tmin = tmat.min(axis=(0, 1))
    tmax = tmat.max(axis=(0, 1))
    spans = tmax - tmin + 1
    ws = np.maximum(((spans + 7) // 8) * 8, 16)
    c0s = np.clip(tmin, 0, C - ws)
    assert (tmax < c0s + ws).all(), (
        f"target band wider than window: spans {spans.max()}"
    )
    return xs, [t.astype(np.float32) for t in ts], \
        tuple(int(v) for v in c0s), tuple(int(v) for v in ws)


def kernel(input, target):
    global LAST_RESULTS
    x = np.asarray(input, dtype=np.float32)
    tg = np.asarray(target).astype(np.int64)
    assert x.shape == (N_TOTAL, C), x.shape
    assert tg.shape == (N_TOTAL,), tg.shape

    xs, ts, c0s, ws = _prep(x, tg)
    key = (c0s, ws)
    if key not in _NC_CACHE:
        _NC_CACHE[key] = build_program(c0s, ws)
    nc = _NC_CACHE[key]

    ci_host = np.ascontiguousarray(
        np.broadcast_to(np.arange(C, dtype=np.float32), (P, C))
    )
    in_maps = [
        {"x": xs[c], "tc": ts[c], "ci": ci_host} for c in range(N_CORES)
    ]
    res = run_bass_kernel_spmd(nc, in_maps, core_ids=list(range(N_CORES)))
    LAST_RESULTS = res
    total = np.float32(0.0)
    for r in res.results:
        total += np.float32(r["out"].reshape(()))
    return np.asarray(total, dtype=np.float32)


if __name__ == "__main__":
    rng = np.random.default_rng(0)
    xs = rng.standard_normal((N_TOTAL, C), dtype=np.float32)
    ts = rng.integers(0, C, size=(N_TOTAL,)).astype(np.int64)
    got = kernel(xs, ts)
    m = np.where(np.arange(C)[None, :] == ts[:, None], xs, -xs)
    hinge = np.maximum(0.0, 1.0 - m)
    loss = np.where(m >= -1.0, hinge * hinge, -4.0 * m)
    want = loss.sum(dtype=np.float64) / N_TOTAL
    print("got", got, "want", want, "rel", abs(got - want) / abs(want))
